# revision 1
# baseline (speedup 1.0000x reference)
"""Causal single-head attention (B=4, S=4096, D=1024) on 8 TRN2 NeuronCores.

Sharding: core = (batch b, half h).  Each core computes attention output for
2048 queries of one batch: query chunks {0,3,4,7} (h=0) or {1,2,5,6} (h=1) of
8x512, which balances causal work.  Each core projects K^T/V for its full
batch (Q projections zippered in between the chunks); K^T lives in SBUF as
four independently-gated fp16 tiles and V is streamed from a DRAM scratch on
the Scalar HWDGE queue.
Scores are computed in the S^T = [k, q] layout so no on-device transposes are
needed anywhere:
  K^T/Q^T/V projections:  psum = sum_d WT[d,:128].T @ x^T[d,:]      (fp16)
  scores^T[k,q]        :  psum = sum_o KT[o,k128].T @ QT[o,q512]    (fp16)
  P = exp(s*scale) * causal_mask   (mask = (iota_k - iota_q) <= a[slot,j])
  den[1,q]             :  ones[k,1].T @ P^T                         (fp16)
  ctx^T[o,q]           :  psum = sum_k V[k,o128].T @ P^T[k,q]       (fp16)
  out = ctx^T * (1/den)  broadcast via ones[1,128].T @ recip[1,q]
"""

import sys

for _p in ("/opt/trn_rl_repo",):
    if _p not in sys.path:
        sys.path.insert(0, _p)

import numpy as np

B, S, D = 4, 4096, 1024
P = 128
CH = 512                       # query chunk
NSLOT = 4                      # chunks per core
NQ = NSLOT * CH                # queries per core
NK = [8, 16, 24, 32]           # k-tiles per slot (uniform across cores)
SLOTBASE = [0, 8, 24, 48]      # amat column base per slot
CHUNKS_H = [[0, 3, 4, 7], [1, 2, 5, 6]]
SCALE = 1.0 / 32.0             # 1/sqrt(D)

_PROGRAM = None


def _build_program():
    import concourse.bass as bass
    import concourse.tile as tile
    import concourse.mybir as mybir
    from concourse import bacc
    from concourse.bass import ds, ts

    f32 = mybir.dt.float32
    f16 = mybir.dt.float16

    nc = bacc.Bacc(trn_type="TRN2", target_bir_lowering=False, debug=False,
                   num_devices=8)

    xT = nc.declare_dram_parameter("xT", [8, P, 8, CH], f16, isOutput=False)
    xqT = nc.declare_dram_parameter("xqT", [NSLOT, P, 8, CH], f16, isOutput=False)
    wqT = nc.declare_dram_parameter("wqT", [P, 8, D], f16, isOutput=False)
    wkT = nc.declare_dram_parameter("wkT", [P, 8, D], f16, isOutput=False)
    wvT = nc.declare_dram_parameter("wvT", [P, 8, D], f16, isOutput=False)
    amat = nc.declare_dram_parameter("amat", [P, 80], f16, isOutput=False)
    dmat = nc.declare_dram_parameter("dmat", [P, CH], f16, isOutput=False)
    ones_k = nc.declare_dram_parameter("ones_k", [P, 1], f16, isOutput=False)
    ones_r = nc.declare_dram_parameter("ones_r", [1, P], f32, isOutput=False)
    outT = nc.declare_dram_parameter("outT", [D, NQ], f32, isOutput=True)

    H = S // 4  # 1024: columns per resident K^T piece
    vscr = nc.dram_tensor("v_scratch", [S, D], f16)

    Exp = mybir.ActivationFunctionType.Exp
    is_le = mybir.AluOpType.is_le
    mult = mybir.AluOpType.mult

    with tile.TileContext(nc, pool_alloc_mode="queue") as tc:
        with (
            tc.tile_pool(name="kt", bufs=1) as kt_pool,
            tc.tile_pool(name="qt", bufs=1) as qt_pool,
            tc.tile_pool(name="const", bufs=1) as const_pool,
        ):
            KTp = [
                kt_pool.tile([P, 8, H], f16, tag=f"kt{i}", name=f"KTp{i}")
                for i in range(4)
            ]
            QTs = [
                qt_pool.tile([P, 8, CH], f16, tag=f"qt{i}", name=f"QTs{i}")
                for i in range(NSLOT)
            ]
            dmat_sb = const_pool.tile([P, CH], f16, tag="dmat")
            amat_sb = const_pool.tile([P, 80], f16, tag="amat")
            ones_k_sb = const_pool.tile([P, 1], f16, tag="onesk")
            ones_r_sb = const_pool.tile([1, P], f32, tag="onesr")
            nc.sync.dma_start(out=dmat_sb[:], in_=dmat[:])
            nc.sync.dma_start(out=amat_sb[:], in_=amat[:])
            nc.sync.dma_start(out=ones_k_sb[:], in_=ones_k[:])
            nc.sync.dma_start(out=ones_r_sb[:], in_=ones_r[:])

            # ---------- Phase 0+1: local projections (K, V, Q zippered) ----
            with (
                tc.tile_pool(name="w0", bufs=1) as w_pool,
                tc.tile_pool(name="xc", bufs=3) as x_pool,
                tc.tile_pool(name="xq", bufs=3) as xq_pool,
                tc.tile_pool(name="vb", bufs=3) as vb_pool,
                tc.tile_pool(name="ps0", bufs=4, space="PSUM") as ps_pool,
            ):
                wk = w_pool.tile([P, 8, D], f16, tag="wk")
                wv = w_pool.tile([P, 8, D], f16, tag="wv")
                wq = w_pool.tile([P, 8, D], f16, tag="wq")
                for half in range(2):
                    nc.sync.dma_start(
                        out=wk[:, :, ds(half * CH, CH)],
                        in_=wkT[:, :, ds(half * CH, CH)],
                    )

                def load_xq(c):
                    xq = xq_pool.tile([P, 8, CH], f16, tag="xq", name=f"xq{c}")
                    nc.scalar.dma_start(
                        out=xq[:],
                        in_=xqT[c],
                    )
                    return xq

                xq_pending = []

                def proj_q(slot):
                    xq = xq_pending[slot]
                    for o in range(8):
                        ps = ps_pool.tile([P, CH], f32, tag="ps", name="psq")
                        for d in range(8):
                            nc.tensor.matmul(
                                ps[:],
                                lhsT=wq[:, d, ts(o, P)],
                                rhs=xq[:, d, :],
                                start=(d == 0),
                                stop=(d == 7),
                            )
                        nc.vector.tensor_copy(QTs[slot][:, o, :], ps[:])

                for c in range(8):
                    xc = x_pool.tile([P, 8, CH], f16, tag="xc", name=f"xc{c}")
                    nc.sync.dma_start(
                        out=xc[:],
                        in_=xT[c],
                    )
                    for o in range(8):
                        ps = ps_pool.tile([P, CH], f32, tag="ps", name="psk")
                        for d in range(8):
                            nc.tensor.matmul(
                                ps[:],
                                lhsT=wk[:, d, ts(o, P)],
                                rhs=xc[:, d, :],
                                start=(d == 0),
                                stop=(d == 7),
                            )
                        nc.vector.tensor_copy(
                            KTp[c // 2][:, o, ds((c % 2) * CH, CH)], ps[:]
                        )
                    if c == 0:
                        # deferred loads: SP/ACT reach these only after the
                        # first chunk's copies, leaving full DMA bandwidth to
                        # the critical wk+xc0 at kernel start
                        nc.sync.dma_start(
                            out=wv[:], in_=wvT[:]
                        )
                        nc.scalar.dma_start(
                            out=wq[:], in_=wqT[:]
                        )
                        xq_pending.append(load_xq(0))
                        xq_pending.append(load_xq(1))
                    for kt_i in range(4):
                        vb = vb_pool.tile([P, D], f16, tag="vb", name="vb")
                        for oh in range(2):
                            ps = ps_pool.tile([P, CH], f32, tag="ps", name="psv")
                            for d in range(8):
                                nc.tensor.matmul(
                                    ps[:],
                                    lhsT=xc[:, d, ts(kt_i, P)],
                                    rhs=wv[:, d, ts(oh, CH)],
                                    start=(d == 0),
                                    stop=(d == 7),
                                )
                            nc.scalar.copy(vb[:, ts(oh, CH)], ps[:])
                        nc.sync.dma_start(
                            out=vscr[ds(c * CH + kt_i * P, P), :], in_=vb[:]
                        )
                    if 1 <= c <= 4:
                        proj_q(c - 1)
                        if c <= 2:
                            xq_pending.append(load_xq(c + 1))

            # ---------------- Phase 2: attention ---------------------------
            with (
                tc.tile_pool(name="ctx", bufs=2) as ctx_pool,
                tc.tile_pool(name="vt", bufs=12) as v_pool,
                tc.tile_pool(name="pt", bufs=12) as p_pool,
                tc.tile_pool(name="et", bufs=3) as e_pool,
                tc.tile_pool(name="fo", bufs=3) as f_pool,
                tc.tile_pool(name="dsb", bufs=2) as den_pool,
                tc.tile_pool(name="pss", bufs=3, space="PSUM") as s_ps_pool,
                tc.tile_pool(name="psc", bufs=3, space="PSUM") as c_ps_pool,
                tc.tile_pool(name="psd", bufs=1, space="PSUM") as d_ps_pool,
                tc.tile_pool(name="psb", bufs=1, space="PSUM") as b_ps_pool,
            ):
                for slot in range(NSLOT):
                    nk = NK[slot]
                    ctx = ctx_pool.tile([P, 8, CH], f32, tag="ctx", name="ctx")
                    den = den_pool.tile([1, CH], f32, tag="den", name="den")
                    for blk in range(nk // 4):
                        p_tiles = []
                        v_tiles = []
                        for j4 in range(4):
                            j = blk * 4 + j4
                            vt = v_pool.tile([P, D], f16, tag="vt", name="vt")
                            nc.scalar.dma_start(out=vt[:], in_=vscr[ds(j * P, P), :])
                            sps = s_ps_pool.tile([P, CH], f32, name="sps")
                            for o in range(8):
                                nc.tensor.matmul(
                                    sps[:],
                                    lhsT=KTp[j // 8][:, o, ds((j % 8) * P, P)],
                                    rhs=QTs[slot][:, o, :],
                                    start=(o == 0),
                                    stop=(o == 7),
                                )
                            et = e_pool.tile([P, CH], f16, tag="et", name="et")
                            nc.scalar.activation(et[:], sps[:], Exp, scale=SCALE)
                            pt = p_pool.tile([P, CH], f16, tag="pt", name="pt")
                            col = SLOTBASE[slot] + j
                            nc.vector.scalar_tensor_tensor(
                                out=pt[:],
                                in0=dmat_sb[:],
                                scalar=amat_sb[:, ds(col, 1)],
                                in1=et[:],
                                op0=is_le,
                                op1=mult,
                            )
                            p_tiles.append(pt)
                            v_tiles.append(vt)
                        dps = d_ps_pool.tile([1, CH], f32, name="dps")
                        for j4 in range(4):
                            nc.tensor.matmul(
                                dps[:],
                                lhsT=ones_k_sb[:],
                                rhs=p_tiles[j4][:],
                                start=(j4 == 0),
                                stop=(j4 == 3),
                            )
                        if blk == 0:
                            nc.vector.tensor_copy(den[:], dps[:])
                        else:
                            nc.vector.tensor_add(den[:], den[:], dps[:])
                        for o in range(8):
                            cps = c_ps_pool.tile([P, CH], f32, name="cps")
                            for j4 in range(4):
                                nc.tensor.matmul(
                                    cps[:],
                                    lhsT=v_tiles[j4][:, ts(o, P)],
                                    rhs=p_tiles[j4][:],
                                    start=(j4 == 0),
                                    stop=(j4 == 3),
                                )
                            if blk == 0:
                                nc.vector.tensor_copy(ctx[:, o, :], cps[:])
                            else:
                                nc.vector.tensor_add(
                                    ctx[:, o, :], ctx[:, o, :], cps[:]
                                )
                    bps = b_ps_pool.tile([P, CH], f32, name="bps")
                    nc.tensor.matmul(
                        bps[:], lhsT=ones_r_sb[:], rhs=den[:], start=True, stop=True
                    )
                    rec = f_pool.tile([P, CH], f32, tag="rec", name="rec")
                    nc.vector.reciprocal(rec[:], bps[:])
                    for o in range(8):
                        ft = f_pool.tile([P, CH], f32, tag="ft", name="ft")
                        nc.vector.tensor_mul(ft[:], ctx[:, o, :], rec[:])
                        nc.sync.dma_start(
                            out=outT[ds(o * P, P), ts(slot, CH)], in_=ft[:]
                        )

    nc.compile()
    return nc


def _get_program():
    global _PROGRAM
    if _PROGRAM is None:
        _PROGRAM = _build_program()
    return _PROGRAM


def _make_in_maps(x, W_query, W_key, W_value):
    xT = np.ascontiguousarray(
        np.asarray(x, dtype=np.float32).transpose(0, 2, 1).astype(np.float16)
    )

    def tile_w(w):
        # [d, o] -> [p, d_slab, o]
        wt = np.asarray(w, dtype=np.float32).T.astype(np.float16)
        return np.ascontiguousarray(wt.reshape(8, P, D).transpose(1, 0, 2))

    def tile_x(xt, nch):
        # [d, s] -> [chunk, p, d_slab, s_off]
        return np.ascontiguousarray(
            xt.reshape(8, P, nch, CH).transpose(2, 1, 0, 3)
        )

    wqT = tile_w(W_query)
    wkT = tile_w(W_key)
    wvT = tile_w(W_value)
    dmat = (
        np.arange(P, dtype=np.float32)[:, None] - np.arange(CH, dtype=np.float32)[None, :]
    )
    dmat = np.ascontiguousarray(dmat.astype(np.float16))
    amat_h = []
    for h in range(2):
        a = np.zeros((P, 80), np.float16)
        for slot in range(NSLOT):
            cid = CHUNKS_H[h][slot]
            for j in range(NK[slot]):
                a[:, SLOTBASE[slot] + j] = CH * cid - P * j
        amat_h.append(a)
    ones_k = np.ones((P, 1), np.float16)
    ones_r = np.ones((1, P), np.float32)

    in_maps = []
    for core in range(8):
        b, h = core // 2, core % 2
        xq_cols = np.concatenate(
            [np.arange(c * CH, (c + 1) * CH) for c in CHUNKS_H[h]]
        )
        xqT_b = tile_x(np.ascontiguousarray(xT[b][:, xq_cols]), NSLOT)
        in_maps.append(
            {
                "xT": tile_x(xT[b], 8),
                "xqT": xqT_b,
                "wqT": wqT,
                "wkT": wkT,
                "wvT": wvT,
                "amat": amat_h[h],
                "dmat": dmat,
                "ones_k": ones_k,
                "ones_r": ones_r,
            }
        )
    return in_maps


def _assemble(results):
    out = np.empty((B, S, D), np.float32)
    for core in range(8):
        b, h = core // 2, core % 2
        oT = np.asarray(results[core]["outT"])  # [D, NQ]
        for slot, c in enumerate(CHUNKS_H[h]):
            out[b, c * CH : (c + 1) * CH, :] = oT[:, slot * CH : (slot + 1) * CH].T
    return out


def run(inputs, trace=False, trace_cores=None):
    """Run the kernel; returns (output, BassKernelResults)."""
    from concourse.bass_utils import run_bass_kernel_spmd

    nc = _get_program()
    in_maps = _make_in_maps(
        inputs["x"], inputs["W_query"], inputs["W_key"], inputs["W_value"]
    )
    kw = {}
    if trace:
        kw = dict(trace=True, trace_cores=trace_cores, stitch_traces=False)
    res = run_bass_kernel_spmd(nc, in_maps, list(range(8)), **kw)
    return _assemble(res.results), res


def kernel(x, W_query, W_key, W_value):
    out, _ = run({"x": x, "W_query": W_query, "W_key": W_key, "W_value": W_value})
    return out



# revision 11
# speedup vs baseline: 1.7262x; 1.7262x over previous
"""Causal single-head attention (B=4, S=4096, D=1024) on 8 TRN2 NeuronCores.

Sharding: core = (batch b, half h).  Each core computes attention output for
2048 queries of one batch: query chunks {0,3,4,7} (h=0) or {1,2,5,6} (h=1) of
8x512, which balances causal work.  Each core projects K^T/V for its full
batch; everything stays SBUF-resident (no DRAM scratch).

Mixed precision (validated against the reference in numpy, rel ~2.4e-3):
  - fp16 island: scores for (q<512, k<512); V/P for keys<512; K/Q/V
    projections feeding those.  Protects the early (few-key) rows where
    softmax averaging is weak.
  - everything else: fp8 e4m3 with DoubleRow matmuls (2x PE throughput).
Scale folding (dodges e4m3 subnormals/overflow):
  Wq8,Wk8 scaled x32 -> s8 = 1024*s -> exp scale 1/32768
  Wv8 scaled x16; P8 stored as p/16 via exp bias -ln(16); den repaired with
  ones8=16; fp16 paths are true-scale.
Layouts (all SBUF):
  K^T  : KT8 4x[P,8,1024] f8 (keys j//8*1024+..), KT16 [P,8,512] f16
  Q^T  : Q8 4x[P,8,512] f8 per slot, Q16 [P,8,512] f16 (slot 0)
  V    : V16 [P,4,1024] f16 (tiles 0-3), V8 [P,28,1024] f8 (tiles 4-31)
  P    : P16 [P,4,512] f16 (tiles 0-3), P8 pairs [P,2,512] f8 (tiles 4+)
  scores^T = [k, q]: psum = sum_d KT[d,k128].T @ QT[d,q512] (no transposes)
"""

import math
import sys

for _p in ("/opt/trn_rl_repo",):
    if _p not in sys.path:
        sys.path.insert(0, _p)

import numpy as np
import ml_dtypes

B, S, D = 4, 4096, 1024
P = 128
CH = 512                       # query chunk
NSLOT = 4                      # chunks per core
NQ = NSLOT * CH                # queries per core
NK = [8, 16, 24, 32]           # k-tiles per slot (uniform across cores)
CHUNKS_H = [[0, 3, 4, 7], [1, 2, 5, 6]]
SC16 = 1.0 / 32.0              # 1/sqrt(D)
SC8 = 1.0 / 32768.0            # 1/sqrt(D) / (32*32)
BIAS8 = -math.log(16.0)        # P8 stored as p/16
F8 = ml_dtypes.float8_e4m3

_PROGRAM = None


def _build_program():
    import concourse.bass as bass
    import concourse.tile as tile
    import concourse.mybir as mybir
    from concourse import bacc
    from concourse.bass import ds, ts

    f32 = mybir.dt.float32
    f16 = mybir.dt.float16
    f8 = mybir.dt.float8e4
    DR = mybir.MatmulPerfMode.DoubleRow

    nc = bacc.Bacc(trn_type="TRN2", target_bir_lowering=False, debug=False,
                   num_devices=8)

    xc16_d = nc.declare_dram_parameter("xc16", [P, 8, CH], f16, isOutput=False)
    x8_d = nc.declare_dram_parameter("x8", [7, P, 8, CH], f8, isOutput=False)
    xq16_d = nc.declare_dram_parameter("xq16", [P, 8, CH], f16, isOutput=False)
    xq8_d = nc.declare_dram_parameter("xq8", [3, P, 8, CH], f8, isOutput=False)
    wk16_d = nc.declare_dram_parameter("wk16", [P, 8, D], f16, isOutput=False)
    wv16_d = nc.declare_dram_parameter("wv16", [P, 8, D], f16, isOutput=False)
    wq16_d = nc.declare_dram_parameter("wq16", [P, 8, D], f16, isOutput=False)
    wk8_d = nc.declare_dram_parameter("wk8", [P, 8, D], f8, isOutput=False)
    wv8_d = nc.declare_dram_parameter("wv8", [P, 8, D], f8, isOutput=False)
    wq8_d = nc.declare_dram_parameter("wq8", [P, 8, D], f8, isOutput=False)
    amat_d = nc.declare_dram_parameter("amat", [P, 32], f16, isOutput=False)
    bias8_d = nc.declare_dram_parameter("bias8", [P, 1], f32, isOutput=False)
    dmat_d = nc.declare_dram_parameter("dmat", [P, CH], f16, isOutput=False)
    ones16_d = nc.declare_dram_parameter("ones16", [P, 1], f16, isOutput=False)
    ones8_d = nc.declare_dram_parameter("ones8", [P, 2, 16], f8, isOutput=False)
    onesr_d = nc.declare_dram_parameter("onesr", [1, P], f32, isOutput=False)
    outT = nc.declare_dram_parameter("outT", [D, NQ], f32, isOutput=True)

    Exp = mybir.ActivationFunctionType.Exp
    Copy = mybir.ActivationFunctionType.Copy
    is_le = mybir.AluOpType.is_le
    mult = mybir.AluOpType.mult

    with tile.TileContext(nc, pool_alloc_mode="queue") as tc:
        with (
            tc.tile_pool(name="kt", bufs=1) as kt_pool,
            tc.tile_pool(name="qt", bufs=1) as qt_pool,
            tc.tile_pool(name="vt", bufs=1) as vt_pool,
            tc.tile_pool(name="const", bufs=1) as const_pool,
        ):
            KT8 = [
                kt_pool.tile([P, 8, 1024], f8, tag=f"kt{i}", name=f"KT8_{i}")
                for i in range(4)
            ]
            KT16 = kt_pool.tile([P, 8, CH], f16, tag="kt16", name="KT16")
            Q8 = [
                qt_pool.tile([P, 8, CH], f8, tag=f"qt{i}", name=f"Q8_{i}")
                for i in range(NSLOT)
            ]
            Q16 = qt_pool.tile([P, 8, CH], f16, tag="qt16", name="Q16")
            V16 = vt_pool.tile([P, 4, D], f16, tag="v16", name="V16")
            V8 = vt_pool.tile([P, 28, D], f8, tag="v8", name="V8")
            dmat = const_pool.tile([P, CH], f16, tag="dmat")
            amat = const_pool.tile([P, 32], f16, tag="amat")
            ones16 = const_pool.tile([P, 1], f16, tag="ones16")
            ones8 = const_pool.tile([P, 2, 16], f8, tag="ones8")
            onesr = const_pool.tile([1, P], f32, tag="onesr")
            bias8 = const_pool.tile([P, 1], f32, tag="bias8")
            nc.sync.dma_start(out=dmat[:], in_=dmat_d[:])
            nc.sync.dma_start(out=amat[:], in_=amat_d[:])
            nc.sync.dma_start(out=bias8[:], in_=bias8_d[:])
            nc.sync.dma_start(out=ones16[:], in_=ones16_d[:])
            nc.sync.dma_start(out=ones8[:], in_=ones8_d[:])
            nc.sync.dma_start(out=onesr[:], in_=onesr_d[:])

            # ---------- Phase A: projections ------------------------------
            with (
                tc.tile_pool(name="w16", bufs=2) as w16_pool,
                tc.tile_pool(name="w8", bufs=1) as w8_pool,
                tc.tile_pool(name="x16", bufs=1) as x16_pool,
                tc.tile_pool(name="x8", bufs=3) as x8_pool,
                tc.tile_pool(name="xq8", bufs=3) as xq8_pool,
                tc.tile_pool(name="ps0", bufs=4, space="PSUM") as ps_pool,
            ):
                wk16 = w16_pool.tile([P, 8, D], f16, tag="w16", name="wk16")
                wv16 = w16_pool.tile([P, 8, D], f16, tag="w16", name="wv16")
                wk8 = w8_pool.tile([P, 8, D], f8, tag="wk8")
                wv8 = w8_pool.tile([P, 8, D], f8, tag="wv8")
                wq8 = w8_pool.tile([P, 8, D], f8, tag="wq8")
                xc16 = x16_pool.tile([P, 8, CH], f16, tag="xc16")
                xq16 = x16_pool.tile([P, 8, CH], f16, tag="xq16")

                nc.sync.dma_start(out=wk16[:], in_=wk16_d[:])
                nc.sync.dma_start(out=xc16[:], in_=xc16_d[:])
                nc.sync.dma_start(out=wv16[:], in_=wv16_d[:])
                nc.scalar.dma_start(out=xq16[:], in_=xq16_d[:])

                # fp16 chunk-0 K (dual store: f16 true + f8 x32)
                for o in range(8):
                    ps = ps_pool.tile([P, CH], f32, tag="ps", name="psk16")
                    for d in range(8):
                        nc.tensor.matmul(
                            ps[:],
                            lhsT=wk16[:, d, ts(o, P)],
                            rhs=xc16[:, d, :],
                            start=(d == 0),
                            stop=(d == 7),
                        )
                    nc.vector.tensor_copy(KT16[:, o, :], ps[:])
                    nc.scalar.activation(
                        KT8[0][:, o, ds(0, CH)], ps[:], Copy, scale=32.0
                    )
                # fp16 chunk-0 V
                for kt in range(4):
                    for oh in range(2):
                        ps = ps_pool.tile([P, CH], f32, tag="ps", name="psv16")
                        for d in range(8):
                            nc.tensor.matmul(
                                ps[:],
                                lhsT=xc16[:, d, ts(kt, P)],
                                rhs=wv16[:, d, ts(oh, CH)],
                                start=(d == 0),
                                stop=(d == 7),
                            )
                        nc.scalar.copy(V16[:, kt, ts(oh, CH)], ps[:])
                # fp16 slot-0 Q (dual store); wq16 reuses wk16's ring slot
                wq16 = w16_pool.tile([P, 8, D], f16, tag="w16", name="wq16")
                nc.scalar.dma_start(out=wq16[:], in_=wq16_d[:])
                for o in range(8):
                    ps = ps_pool.tile([P, CH], f32, tag="ps", name="psq16")
                    for d in range(8):
                        nc.tensor.matmul(
                            ps[:],
                            lhsT=wq16[:, d, ts(o, P)],
                            rhs=xq16[:, d, :],
                            start=(d == 0),
                            stop=(d == 7),
                        )
                    nc.vector.tensor_copy(Q16[:, o, :], ps[:])
                    nc.scalar.activation(Q8[0][:, o, :], ps[:], Copy, scale=32.0)

                # fp8 weights (loads overlap the fp16 island compute)
                nc.sync.dma_start(out=wk8[:], in_=wk8_d[:])
                nc.sync.dma_start(out=wv8[:], in_=wv8_d[:])
                nc.scalar.dma_start(out=wq8[:], in_=wq8_d[:])

                # fp8 chunks 1-7: K + V, Q slots zippered after chunks 1-3
                for c in range(1, 8):
                    xc = x8_pool.tile([P, 8, CH], f8, tag="xc", name=f"xc{c}")
                    nc.sync.dma_start(out=xc[:], in_=x8_d[c - 1])
                    xq = None
                    if c <= 3:
                        xq = xq8_pool.tile(
                            [P, 8, CH], f8, tag="xq", name=f"xq{c}"
                        )
                        nc.scalar.dma_start(out=xq[:], in_=xq8_d[c - 1])
                    for o in range(8):
                        ps = ps_pool.tile([P, CH], f32, tag="ps", name="psk8")
                        for d2 in range(4):
                            nc.tensor.matmul(
                                ps[:],
                                lhsT=wk8[:, ds(2 * d2, 2), ts(o, P)],
                                rhs=xc[:, ds(2 * d2, 2), :],
                                start=(d2 == 0),
                                stop=(d2 == 3),
                                perf_mode=DR,
                            )
                        nc.vector.tensor_copy(
                            KT8[c // 2][:, o, ds((c % 2) * CH, CH)], ps[:]
                        )
                    for kt in range(4):
                        for oh in range(2):
                            ps = ps_pool.tile([P, CH], f32, tag="ps", name="psv8")
                            for d2 in range(4):
                                nc.tensor.matmul(
                                    ps[:],
                                    lhsT=xc[:, ds(2 * d2, 2), ts(kt, P)],
                                    rhs=wv8[:, ds(2 * d2, 2), ts(oh, CH)],
                                    start=(d2 == 0),
                                    stop=(d2 == 3),
                                    perf_mode=DR,
                                )
                            nc.scalar.copy(
                                V8[:, 4 * c + kt - 4, ts(oh, CH)], ps[:]
                            )
                    if c <= 3:
                        for o in range(8):
                            ps = ps_pool.tile([P, CH], f32, tag="ps", name="psq8")
                            for d2 in range(4):
                                nc.tensor.matmul(
                                    ps[:],
                                    lhsT=wq8[:, ds(2 * d2, 2), ts(o, P)],
                                    rhs=xq[:, ds(2 * d2, 2), :],
                                    start=(d2 == 0),
                                    stop=(d2 == 3),
                                    perf_mode=DR,
                                )
                            nc.vector.tensor_copy(Q8[c][:, o, :], ps[:])

            # ---------------- Phase B: attention --------------------------
            with (
                tc.tile_pool(name="p16", bufs=2) as p16_pool,
                tc.tile_pool(name="p8", bufs=24) as p8_pool,
                tc.tile_pool(name="et", bufs=4) as e_pool,
                tc.tile_pool(name="fo", bufs=6) as f_pool,
                tc.tile_pool(name="dsb", bufs=2) as den_pool,
                tc.tile_pool(name="pss", bufs=3, space="PSUM") as s_ps_pool,
                tc.tile_pool(name="psc", bufs=2, space="PSUM") as c_ps_pool,
                tc.tile_pool(name="psd", bufs=1, space="PSUM") as d_ps_pool,
                tc.tile_pool(name="psb", bufs=1, space="PSUM") as b_ps_pool,
            ):
                for s in range(NSLOT):
                    nk = NK[s]
                    np8 = (nk - 4) // 2
                    P16 = p16_pool.tile([P, 4, CH], f16, tag="p16", name="P16")
                    P8 = [
                        p8_pool.tile([P, 2, CH], f8, tag="p8", name=f"P8_{s}_{t}")
                        for t in range(np8)
                    ]
                    for j in range(nk):
                        mm16 = (s == 0 and j < 4)
                        sps = s_ps_pool.tile([P, CH], f32, name="sps")
                        if mm16:
                            for o in range(8):
                                nc.tensor.matmul(
                                    sps[:],
                                    lhsT=KT16[:, o, ds(j * P, P)],
                                    rhs=Q16[:, o, :],
                                    start=(o == 0),
                                    stop=(o == 7),
                                )
                        else:
                            for d2 in range(4):
                                nc.tensor.matmul(
                                    sps[:],
                                    lhsT=KT8[j // 8][
                                        :, ds(2 * d2, 2), ds((j % 8) * P, P)
                                    ],
                                    rhs=Q8[s][:, ds(2 * d2, 2), :],
                                    start=(d2 == 0),
                                    stop=(d2 == 3),
                                    perf_mode=DR,
                                )
                        scale = SC16 if mm16 else SC8
                        bias = 0.0 if j < 4 else bias8[:]
                        if j < 4:
                            dst = P16[:, j, :]
                        else:
                            dst = P8[(j - 4) // 2][:, (j - 4) % 2, :]
                        stt = (s == 0) or (j >= nk - 8)
                        if stt:
                            et = e_pool.tile([P, CH], f16, tag="et", name="et")
                            nc.scalar.activation(
                                et[:], sps[:], Exp, scale=scale, bias=bias
                            )
                            col = j if s == 0 else 8 * s + (j - (nk - 8))
                            nc.vector.scalar_tensor_tensor(
                                out=dst,
                                in0=dmat[:],
                                scalar=amat[:, ds(col, 1)],
                                in1=et[:],
                                op0=is_le,
                                op1=mult,
                            )
                        else:
                            nc.scalar.activation(
                                dst, sps[:], Exp, scale=scale, bias=bias
                            )
                    # denominator
                    dps = d_ps_pool.tile([1, CH], f32, name="dps")
                    for j in range(4):
                        nc.tensor.matmul(
                            dps[:],
                            lhsT=ones16[:],
                            rhs=P16[:, j, :],
                            start=(j == 0),
                            stop=False,
                        )
                    for t in range(np8):
                        nc.tensor.matmul(
                            dps[:],
                            lhsT=ones8[:, :, ds(0, 1)],
                            rhs=P8[t][:],
                            start=False,
                            stop=(t == np8 - 1),
                            perf_mode=DR,
                        )
                    den = den_pool.tile([1, CH], f32, tag="den", name="den")
                    nc.vector.tensor_copy(den[:], dps[:])
                    bps = b_ps_pool.tile([P, CH], f32, name="bps")
                    nc.tensor.matmul(
                        bps[:], lhsT=onesr[:], rhs=den[:], start=True, stop=True
                    )
                    rec = f_pool.tile([P, CH], f32, tag="rec", name="rec")
                    nc.vector.reciprocal(rec[:], bps[:])
                    # context
                    for o in range(8):
                        cps = c_ps_pool.tile([P, CH], f32, name="cps")
                        for j in range(4):
                            nc.tensor.matmul(
                                cps[:],
                                lhsT=V16[:, j, ts(o, P)],
                                rhs=P16[:, j, :],
                                start=(j == 0),
                                stop=False,
                            )
                        for t in range(np8):
                            nc.tensor.matmul(
                                cps[:],
                                lhsT=V8[:, ds(2 * t, 2), ts(o, P)],
                                rhs=P8[t][:],
                                start=False,
                                stop=(t == np8 - 1),
                                perf_mode=DR,
                            )
                        ft = f_pool.tile([P, CH], f32, tag="ft", name="ft")
                        nc.vector.tensor_mul(ft[:], cps[:], rec[:])
                        nc.sync.dma_start(
                            out=outT[ds(o * P, P), ts(s, CH)], in_=ft[:]
                        )

    nc.compile()
    return nc


def _get_program():
    global _PROGRAM
    if _PROGRAM is None:
        _PROGRAM = _build_program()
    return _PROGRAM


def _tile_w(w, scale, dtype):
    # [o, i] -> [p, d_slab, o] with d = d_slab*128 + p
    wt = (np.asarray(w, dtype=np.float32).T * scale).astype(dtype)
    return np.ascontiguousarray(wt.reshape(8, P, D).transpose(1, 0, 2))


def _tile_x(xt, dtype):
    # [d, s_cols] -> [p, d_slab, s] (or [n, p, d_slab, s] for multi-chunk)
    ncols = xt.shape[1]
    nch = ncols // CH
    t = np.ascontiguousarray(
        xt.reshape(8, P, nch, CH).transpose(2, 1, 0, 3)
    ).astype(dtype)
    if nch == 1:
        return np.ascontiguousarray(t[0])
    return t


def _make_in_maps(x, W_query, W_key, W_value):
    xT = np.asarray(x, dtype=np.float32).transpose(0, 2, 1)  # [B, D, S]

    wk16 = _tile_w(W_key, 1.0, np.float16)
    wv16 = _tile_w(W_value, 1.0, np.float16)
    wq16 = _tile_w(W_query, 1.0, np.float16)
    wk8 = _tile_w(W_key, 32.0, F8)
    wv8 = _tile_w(W_value, 16.0, F8)
    wq8 = _tile_w(W_query, 32.0, F8)

    dmat = (
        np.arange(P, dtype=np.float32)[:, None]
        - np.arange(CH, dtype=np.float32)[None, :]
    ).astype(np.float16)
    amat_h = []
    for h in range(2):
        a = np.full((P, 32), -32768.0, np.float32)
        for sl in range(NSLOT):
            cid = CHUNKS_H[h][sl]
            nk = NK[sl]
            if sl == 0:
                for j in range(8):
                    a[:, j] = CH * cid - P * j
            else:
                for j in range(nk - 8, nk):
                    a[:, 8 * sl + (j - (nk - 8))] = CH * cid - P * j
        amat_h.append(np.ascontiguousarray(a.astype(np.float16)))
    ones16 = np.ones((P, 1), np.float16)
    ones8 = np.full((P, 2, 16), 16.0, F8)
    onesr = np.ones((1, P), np.float32)
    bias8 = np.full((P, 1), BIAS8, np.float32)

    in_maps = []
    for core in range(8):
        b, h = core // 2, core % 2
        xb = xT[b]
        q0 = CHUNKS_H[h][0]
        xq_cols = np.concatenate(
            [np.arange(c * CH, (c + 1) * CH) for c in CHUNKS_H[h][1:]]
        )
        in_maps.append(
            {
                "xc16": _tile_x(xb[:, :CH], np.float16),
                "x8": _tile_x(xb[:, CH:], F8),
                "xq16": _tile_x(xb[:, q0 * CH : (q0 + 1) * CH], np.float16),
                "xq8": _tile_x(np.ascontiguousarray(xb[:, xq_cols]), F8),
                "wk16": wk16,
                "wv16": wv16,
                "wq16": wq16,
                "wk8": wk8,
                "wv8": wv8,
                "wq8": wq8,
                "amat": amat_h[h],
                "dmat": dmat,
                "ones16": ones16,
                "ones8": ones8,
                "onesr": onesr,
                "bias8": bias8,
            }
        )
    return in_maps


def _assemble(results):
    out = np.empty((B, S, D), np.float32)
    for core in range(8):
        b, h = core // 2, core % 2
        oT = np.asarray(results[core]["outT"])  # [D, NQ]
        for slot, c in enumerate(CHUNKS_H[h]):
            out[b, c * CH : (c + 1) * CH, :] = oT[:, slot * CH : (slot + 1) * CH].T
    return out


def run(inputs, trace=False, trace_cores=None):
    """Run the kernel; returns (output, BassKernelResults)."""
    from concourse.bass_utils import run_bass_kernel_spmd

    nc = _get_program()
    in_maps = _make_in_maps(
        inputs["x"], inputs["W_query"], inputs["W_key"], inputs["W_value"]
    )
    kw = {}
    if trace:
        kw = dict(trace=True, trace_cores=trace_cores, stitch_traces=False)
    res = run_bass_kernel_spmd(nc, in_maps, list(range(8)), **kw)
    return _assemble(res.results), res


def kernel(x, W_query, W_key, W_value):
    out, _ = run({"x": x, "W_query": W_query, "W_key": W_key, "W_value": W_value})
    return out
